# revision 1
# baseline (speedup 1.0000x reference)
"""AttentionMIL pooling kernel for 8 Trainium2 NeuronCores.

Math (per slide b): h = tanh(X @ W1^T); s = h @ w2; a = softmax(s);
out = a^T @ X, with X [N=8192, D=1024], W1 [H=256, D], w2 [H].

Strategy (v2 — single X copy, wsum on the vector engine):
  - Data-parallel over the slide dim: 16 slides / 8 cores = 2 per core.
  - ONE host-swizzled transposed copy of X per core (bf16, [128(d-chunk),
    n-free] tiles) — 32 MiB of HBM traffic per core instead of the 64 MiB
    the two-layout variant needed.
  - Scores in h^T orientation: for each 512-row n-tile, PE computes
    hT[half] [128, 512] = w1t_chunk^T @ xt_chunk accumulated over the 8
    d-chunks (N=512 moving operand — best PE efficiency), ACT applies
    tanh -> bf16, then two more PE matmuls with a REPLICATED w2 stationary
    ([128, 128] with every column equal to w2-half) produce the scores
    already broadcast across all 128 partitions; ACT exp -> e128 bf16.
  - Softmax without a max pass: s = w2 . tanh(.) is bounded by ||w2||_1
    (~13 for this data), so exp(s) cannot overflow fp32.
  - Weighted sum OFF the tensor engine, split DVE+Scalar: per tile, 5 of
    the 8 d-chunks use DVE scalar_tensor_tensor (fused mul + accum_out
    free-dim reduce, 1x rate); the other 3 chunks are multiplied in one
    2x-rate DVE tensor_mul (e broadcast via stride-0 dim) and reduced on
    the Scalar engine via Copy-with-accum_out. Per-tile partial sums land
    in disjoint columns of per-engine SBUF accumulators (shared tiles
    would thread false cross-engine deps); one final reduce folds them.
    GpSimd is kept out of the hot loop (its tensor ops poison DVE ~2.5x
    via SBUF contention, measured).
  - l = sum(e): each tile's score row is DMA'd out (1 KiB) and summed on
    host; out = acc / l on host (tiny).
  - Pipeline: scores lag the h-matmuls by one tile, the wsum batch by
    two, ACT accums by three; on the Scalar queue exp(g-1) precedes
    tanh(g) so DVE never stalls behind tanh.
"""

import sys

sys.path.insert(0, "/opt/trn_rl_repo")

import numpy as np
import ml_dtypes

import concourse.bacc as bacc
import concourse.tile as tile
from concourse import mybir
from concourse.bass_utils import run_bass_kernel_spmd

BF16 = ml_dtypes.bfloat16
B, N, D, H = 16, 8192, 1024, 256
NCORES = 8
SPC = B // NCORES          # slides per core
NT = 512                   # rows of N per tile
TILES = N // NT
KCH = D // 128             # d-chunks (contraction chunks of 128)
HH = H // 128              # h halves
GP_CHUNKS = 3              # d-chunks routed DVE(mul) + Scalar(accum-reduce)

_NC_CACHE = {}


def _build_nc():
    bf = mybir.dt.bfloat16
    f32 = mybir.dt.float32
    AF = mybir.ActivationFunctionType
    OP = mybir.AluOpType

    nc = bacc.Bacc("TRN2", num_devices=NCORES)
    # Host-swizzled transposed layout: each per-tile DMA reads one fully
    # contiguous 1 MiB region into a [128, free] SBUF tile.
    #   xt[s, t, p, k*NT + j] = X[s, t*NT + j, k*128 + p]
    xt = nc.declare_dram_parameter("xt", [SPC, TILES, 128, KCH * NT], bf, isOutput=False)
    # w1t[p, k*H + h] = W1[h, k*128 + p]
    w1t = nc.declare_dram_parameter("w1t", [128, KCH * H], bf, isOutput=False)
    # w2rep[p, half*128 + c] = W2[0, half*128 + p]  (replicated along c)
    w2rep = nc.declare_dram_parameter("w2rep", [128, H], bf, isOutput=False)
    outp = nc.declare_dram_parameter("out", [SPC, 128, KCH], f32, isOutput=True)
    # e row per tile, summed on host for the softmax denominator
    oute = nc.declare_dram_parameter("oute", [SPC, TILES, NT], bf, isOutput=True)

    with tile.TileContext(nc) as tc:
        with tc.tile_pool(name="const", bufs=1) as constp, \
             tc.tile_pool(name="xt", bufs=6) as xtp, \
             tc.tile_pool(name="tanh", bufs=3) as tp, \
             tc.tile_pool(name="e128", bufs=4) as ep, \
             tc.tile_pool(name="scr", bufs=2) as scrp, \
             tc.tile_pool(name="scra", bufs=2) as scrap, \
             tc.tile_pool(name="gprod", bufs=4) as gprodp, \
             tc.tile_pool(name="racc", bufs=2) as raccp, \
             tc.tile_pool(name="outsb", bufs=2) as outsbp, \
             tc.tile_pool(name="hps", bufs=2, space="PSUM") as hpsp, \
             tc.tile_pool(name="sps", bufs=2, space="PSUM") as spsp, \
             tc.tile_pool(name="warm", bufs=1, space="PSUM") as warmp:

            w1t_sb = constp.tile([128, KCH * H], bf)
            nc.gpsimd.dma_start(w1t_sb[:], w1t[:, :])
            w2r_sb = constp.tile([128, H], bf)
            nc.gpsimd.dma_start(w2r_sb[:], w2rep[:, :])

            warm_sb = constp.tile([128, 256], bf)
            nc.gpsimd.memset(warm_sb[:], 0.0)
            warm_ps = warmp.tile([128, 256], f32)
            # bridge PE activity from preamble until the first xt DMA lands
            # (~18us in): a gap >3.4us re-throttles HAM and the first h-block
            # then runs at half rate (measured 427ns/MM cold vs 216 warm)
            for _ in range(34):
                nc.tensor.matmul(
                    warm_ps[:, 0:H], warm_sb[:, 0:128], warm_sb[:, 0:H],
                    start=True, stop=True, skip_group_check=True,
                )

            state = {}          # per-slide persistent tiles
            prev = None         # (s, t, xt_sb, tanh_sb)
            prev_e = None       # (s, t, xt_sb, e_sb)
            prev_gp = None      # (s, t, prod_g)

            def emit_scores(s, t, xt_sb, tanh_sb):
                # scores: two matmuls with replicated-w2 stationary ->
                # s_ps [128, 512] (every partition = the score row)
                s_ps = spsp.tile([128, NT], f32)
                for half in range(HH):
                    nc.tensor.matmul(
                        s_ps[:],
                        w2r_sb[:, half * 128:(half + 1) * 128],
                        tanh_sb[:, half * NT:(half + 1) * NT],
                        start=(half == 0), stop=(half == HH - 1),
                    )
                e_sb = ep.tile([128, NT], bf)
                nc.scalar.activation(e_sb[:], s_ps[:], AF.Exp)
                nc.sync.dma_start(oute[s, t:t + 1, :], e_sb[0:1, :])
                return (s, t, xt_sb, e_sb)

            def emit_wsum(s, t, xt_sb, e_sb):
                racc_a, racc_d = state[s]
                scr = scrp.tile([128, NT], bf)
                # Fused STT runs at 1x on DVE (bf16 2x is stock-op only), so
                # DVE alone (8 x 604ns) can't keep up with PE (3.84us/tile).
                # For GP_CHUNKS d-chunks DVE only multiplies (2x mode) and
                # the Scalar engine reduces via Copy-with-accum. GpSimd is
                # kept OUT of the loop: concurrent GpSimd tensor ops poison
                # DVE throughput ~2.5x (SBUF contention, measured).
                prod_g = gprodp.tile([128, GP_CHUNKS * NT], bf)
                nc.vector.tensor_mul(
                    prod_g[:].rearrange("p (k j) -> p k j", k=GP_CHUNKS),
                    xt_sb[:, 0:GP_CHUNKS * NT].rearrange(
                        "p (k j) -> p k j", k=GP_CHUNKS
                    ),
                    e_sb[:, None, :].broadcast_to([128, GP_CHUNKS, NT]),
                )
                for k in range(GP_CHUNKS, KCH):
                    kd = k - GP_CHUNKS
                    nc.vector.scalar_tensor_tensor(
                        scr[:],
                        xt_sb[:, k * NT:(k + 1) * NT],
                        1.0,
                        e_sb[:],
                        op0=OP.mult,
                        op1=OP.mult,
                        accum_out=racc_d[:, kd * TILES + t: kd * TILES + t + 1],
                    )
                return (s, t, prod_g)

            def emit_gp_accum(s, t, prod_g):
                racc_a, racc_d = state[s]
                scr_a = scrap.tile([128, NT], bf)
                for k in range(GP_CHUNKS):
                    nc.scalar.activation(
                        scr_a[:],
                        prod_g[:, k * NT:(k + 1) * NT],
                        AF.Copy,
                        accum_out=racc_a[:, k * TILES + t: k * TILES + t + 1],
                    )
                if t == TILES - 1:
                    out_sb = outsbp.tile([128, KCH], f32)
                    nc.vector.reduce_sum(
                        out_sb[:, 0:GP_CHUNKS],
                        racc_a[:].rearrange("p (k t) -> p k t", k=GP_CHUNKS),
                        axis=mybir.AxisListType.X,
                    )
                    nc.vector.reduce_sum(
                        out_sb[:, GP_CHUNKS:KCH],
                        racc_d[:].rearrange("p (k t) -> p k t", k=KCH - GP_CHUNKS),
                        axis=mybir.AxisListType.X,
                    )
                    nc.gpsimd.dma_start(outp[s], out_sb[:])

            for g in range(SPC * TILES):
                s, t = divmod(g, TILES)
                if t == 0:
                    # separate accumulators for the ACT and DVE chunk sets —
                    # a shared tile would thread false cross-engine deps
                    racc_a = raccp.tile(
                        [128, GP_CHUNKS * TILES], f32,
                        tag=f"racca{s}", name=f"racca{s}",
                    )
                    racc_d = raccp.tile(
                        [128, (KCH - GP_CHUNKS) * TILES], f32,
                        tag=f"raccd{s}", name=f"raccd{s}",
                    )
                    state[s] = (racc_a, racc_d)
                xt_sb = xtp.tile([128, KCH * NT], bf)
                nc.sync.dma_start(xt_sb[:], xt[s, t])
                early_e = None
                if g == 1:
                    early_e = emit_scores(*prev)
                h_ps = hpsp.tile([128, HH * NT], f32, name="h_ps")
                for half in range(HH):
                    for k in range(KCH):
                        nc.tensor.matmul(
                            h_ps[:, half * NT:(half + 1) * NT],
                            w1t_sb[:, k * H + half * 128: k * H + half * 128 + 128],
                            xt_sb[:, k * NT:(k + 1) * NT],
                            start=(k == 0), stop=(k == KCH - 1),
                        )
                # Pipeline: scores lag one tile (w2-MMs ride after h(g) on
                # PE; exp(g-1) precedes tanh(g) on ACT so DVE never stalls
                # behind tanh), the DVE wsum batch lags TWO tiles so a full
                # period of ready work buffers it against exp jitter.
                # (g==1: scores(0) already emitted before h(1) to shorten
                # the startup chain.)
                next_e = None
                if prev is not None and g != 1:
                    next_e = emit_scores(*prev)
                elif g == 1:
                    next_e = early_e
                next_gp = None
                if prev_e is not None:
                    next_gp = emit_wsum(*prev_e)
                prev_e = next_e
                tanh_sb = tp.tile([128, HH * NT], bf)
                nc.scalar.activation(tanh_sb[:], h_ps[:], AF.Tanh)
                if prev_gp is not None:
                    emit_gp_accum(*prev_gp)
                prev_gp = next_gp
                prev = (s, t, xt_sb, tanh_sb)
            next_e = emit_scores(*prev)
            next_gp = emit_wsum(*prev_e)
            emit_gp_accum(*prev_gp)
            emit_gp_accum(*next_gp)
            prev_gp = emit_wsum(*next_e)
            emit_gp_accum(*prev_gp)

    nc.compile()
    return nc


def _get_nc():
    if "nc" not in _NC_CACHE:
        _NC_CACHE["nc"] = _build_nc()
    return _NC_CACHE["nc"]


def _prep_inputs(tiles_embeddings, W1, W2):
    X_bf = tiles_embeddings.astype(BF16)
    # xt[b, t, p, k, j] = X[b, t*NT + j, k*128 + p]
    xt_sw = np.ascontiguousarray(
        X_bf.reshape(B, TILES, NT, KCH, 128).transpose(0, 1, 4, 3, 2)
    ).reshape(B, TILES, 128, KCH * NT)
    # w1t[p, k, h] = W1[h, k*128 + p]
    w1t = np.ascontiguousarray(
        W1.astype(BF16).reshape(H, KCH, 128).transpose(2, 1, 0)
    ).reshape(128, KCH * H)
    # w2rep[p, half*128 + c] = W2[0, half*128 + p]
    w2rep = np.ascontiguousarray(
        np.broadcast_to(
            W2.astype(BF16).reshape(HH, 128).transpose(1, 0)[:, :, None],
            (128, HH, 128),
        )
    ).reshape(128, H)
    return [
        {
            "xt": xt_sw[c * SPC:(c + 1) * SPC],
            "w1t": w1t,
            "w2rep": w2rep,
        }
        for c in range(NCORES)
    ]


def _run(tiles_embeddings, W1, W2, **spmd_kwargs):
    nc = _get_nc()
    in_maps = _prep_inputs(tiles_embeddings, W1, W2)
    res = run_bass_kernel_spmd(nc, in_maps, core_ids=list(range(NCORES)), **spmd_kwargs)
    acc = np.concatenate([r["out"] for r in res.results], axis=0)       # [B, 128, KCH]
    e = np.concatenate([r["oute"] for r in res.results], axis=0)        # [B, TILES, NT]
    l = e.astype(np.float64).sum(axis=(1, 2))                           # [B]
    # out[b, k*128 + p] = acc[b, p, k]
    out = acc.transpose(0, 2, 1).reshape(B, D) / l[:, None]
    return out.astype(np.float32, copy=False), res


def kernel(tiles_embeddings, W1, W2):
    out, _ = _run(
        np.asarray(tiles_embeddings), np.asarray(W1), np.asarray(W2)
    )
    return out



# revision 2
# speedup vs baseline: 1.0384x; 1.0384x over previous
"""AttentionMIL pooling kernel for 8 Trainium2 NeuronCores.

Math (per slide b): h = tanh(X @ W1^T); s = h @ w2; a = softmax(s);
out = a^T @ X, with X [N=8192, D=1024], W1 [H=256, D], w2 [H].

Strategy (v3 — v2 pipeline with tighter startup/tail and clean DMA FIFO):
  - Data-parallel over the slide dim: 16 slides / 8 cores = 2 per core.
  - ONE host-swizzled transposed copy of X per core (bf16, [128(d-chunk),
    n-free] tiles) — 32 MiB of HBM traffic per core (DMA floor ~94us).
  - Scores in h^T orientation: for each 512-row n-tile, PE computes
    hT[half] [128, 512] = w1t_chunk^T @ xt_chunk accumulated over the 8
    d-chunks, ACT applies tanh -> bf16, then two PE matmuls with a
    REPLICATED w2 stationary produce the scores broadcast across all 128
    partitions; ACT exp -> e128 bf16.  exp(s) cannot overflow fp32 since
    |s| <= ||w2||_1 (~13 for this data) — no max pass needed.
  - Weighted sum OFF the tensor engine, split DVE+Scalar (5/3): 5 of the
    8 d-chunks use DVE scalar_tensor_tensor (fused mul + accum_out free-
    dim reduce, 1x); the other 3 are multiplied in one 2x-rate DVE
    tensor_mul (e broadcast via stride-0 dim) and reduced on the Scalar
    engine via Copy-with-accum_out.  Partial sums land in disjoint
    columns of per-engine SBUF accumulators (a shared tile would thread
    false cross-engine deps); one final reduce folds them.  GpSimd is
    kept out of the hot loop (its SBUF port is physically shared with
    DVE — tensor ops there poison DVE ~2.5x, measured).
  - l = sum(e): each tile's score row is DMA'd out (1 KiB) and summed on
    host; out = acc / l on host (tiny).
  - v3 scheduling:
      * w1t/w2rep via sync HWDGE FIRST; xt tile 0 split into 4 quarter
        DMAs and tile 1 into halves so the first h-MMs start ~10us in
        instead of ~14.5us; h-MM loop is k-outer so a quarter unblocks
        its 4 MMs.  12 warm matmuls bridge the preamble (HAM re-throttles
        after >3.4us of PE idle; cold MMs run at half rate).
      * lag-1 pipeline: scores(g-1) ride the PE queue right after h(g);
        ACT FIFO order per period is exp(g-1), tanh(g), accums(g-1) so
        DVE unblocks first and PE never waits behind accum copies.
      * oute stores stay on the sync queue but are emitted two tiles
        late, so their e_sb waits are always satisfied and never head-of-
        line-block the next xt dispatch (v2 lost DMA regularity to this).
"""

import sys

sys.path.insert(0, "/opt/trn_rl_repo")

import numpy as np
import ml_dtypes

import concourse.bacc as bacc
import concourse.tile as tile
from concourse import mybir
from concourse.bass_utils import run_bass_kernel_spmd

BF16 = ml_dtypes.bfloat16
B, N, D, H = 16, 8192, 1024, 256
NCORES = 8
SPC = B // NCORES          # slides per core
NT = 512                   # rows of N per tile
TILES = N // NT
KCH = D // 128             # d-chunks (contraction chunks of 128)
HH = H // 128              # h halves
GP_CHUNKS = 3              # d-chunks routed DVE(mul) + Scalar(accum-reduce)
WARM_MMS = 12
OUTE_LAG = 2

_NC_CACHE = {}


def _build_nc():
    bf = mybir.dt.bfloat16
    f32 = mybir.dt.float32
    AF = mybir.ActivationFunctionType
    OP = mybir.AluOpType

    nc = bacc.Bacc("TRN2", num_devices=NCORES)
    # Host-swizzled transposed layout: each per-tile DMA reads one fully
    # contiguous 1 MiB region into a [128, free] SBUF tile.
    #   xt[s, t, p, k*NT + j] = X[s, t*NT + j, k*128 + p]
    xt = nc.declare_dram_parameter("xt", [SPC, TILES, 128, KCH * NT], bf, isOutput=False)
    # w1t[p, k*H + h] = W1[h, k*128 + p]
    w1t = nc.declare_dram_parameter("w1t", [128, KCH * H], bf, isOutput=False)
    # w2rep[p, half*128 + c] = W2[0, half*128 + p]  (replicated along c)
    w2rep = nc.declare_dram_parameter("w2rep", [128, H], bf, isOutput=False)
    outp = nc.declare_dram_parameter("out", [SPC, 128, KCH], f32, isOutput=True)
    # e row per tile, summed on host for the softmax denominator
    oute = nc.declare_dram_parameter("oute", [SPC, TILES, NT], bf, isOutput=True)

    with tile.TileContext(nc) as tc:
        with tc.tile_pool(name="const", bufs=1) as constp, \
             tc.tile_pool(name="xt", bufs=8) as xtp, \
             tc.tile_pool(name="tanh", bufs=3) as tp, \
             tc.tile_pool(name="e128", bufs=4) as ep, \
             tc.tile_pool(name="scr", bufs=2) as scrp, \
             tc.tile_pool(name="scra", bufs=2) as scrap, \
             tc.tile_pool(name="gprod", bufs=4) as gprodp, \
             tc.tile_pool(name="racc", bufs=2) as raccp, \
             tc.tile_pool(name="outsb", bufs=2) as outsbp, \
             tc.tile_pool(name="hps", bufs=2, space="PSUM") as hpsp, \
             tc.tile_pool(name="sps", bufs=2, space="PSUM") as spsp, \
             tc.tile_pool(name="warm", bufs=1, space="PSUM") as warmp:

            # Constants go FIRST on the sync (HWDGE) queue: w1t gates every
            # h-matmul, so nothing may precede it in the DMA FIFO.
            w1t_sb = constp.tile([128, KCH * H], bf)
            nc.sync.dma_start(w1t_sb[:], w1t[:, :])
            w2r_sb = constp.tile([128, H], bf)

            warm_sb = constp.tile([128, 256], bf)
            nc.gpsimd.memset(warm_sb[:], 0.0)
            warm_ps = warmp.tile([128, 256], f32)
            # bridge PE activity from the engine preamble (~6.5us) until the
            # first xt quarter lands (~10us): a gap >3.4us re-throttles HAM
            # and the first h-block then runs at half rate.
            for _ in range(WARM_MMS):
                nc.tensor.matmul(
                    warm_ps[:, 0:H], warm_sb[:, 0:128], warm_sb[:, 0:H],
                    start=True, stop=True, skip_group_check=True,
                )

            state = {}          # per-slide persistent accumulator tiles
            prevT = None        # (s, t, xt_sb, tanh_sb) awaiting scores
            prevE = None        # (s, t, xt_sb, e_sb) awaiting wsum+accums
            pend_oute = []      # [(s, t, e_sb)] oute stores, emitted late

            def emit_scores(s, t, xt_sb, tanh_sb):
                # scores: two matmuls with replicated-w2 stationary ->
                # s_ps [128, 512] (every partition = the score row)
                s_ps = spsp.tile([128, NT], f32)
                for half in range(HH):
                    nc.tensor.matmul(
                        s_ps[:],
                        w2r_sb[:, half * 128:(half + 1) * 128],
                        tanh_sb[:, half * NT:(half + 1) * NT],
                        start=(half == 0), stop=(half == HH - 1),
                    )
                e_sb = ep.tile([128, NT], bf)
                nc.scalar.activation(e_sb[:], s_ps[:], AF.Exp)
                pend_oute.append((s, t, e_sb))
                return (s, t, xt_sb, e_sb)

            def emit_wsum(s, t, xt_sb, e_sb):
                racc_a, racc_d = state[s]
                # mul for the ACT-owned chunks first (2x-rate, one op) so the
                # Scalar engine's accum copies unblock as early as possible.
                prod_g = gprodp.tile([128, GP_CHUNKS * NT], bf)
                nc.vector.tensor_mul(
                    prod_g[:].rearrange("p (k j) -> p k j", k=GP_CHUNKS),
                    xt_sb[:, 0:GP_CHUNKS * NT].rearrange(
                        "p (k j) -> p k j", k=GP_CHUNKS
                    ),
                    e_sb[:, None, :].broadcast_to([128, GP_CHUNKS, NT]),
                )
                scr = scrp.tile([128, NT], bf)
                for k in range(GP_CHUNKS, KCH):
                    kd = k - GP_CHUNKS
                    nc.vector.scalar_tensor_tensor(
                        scr[:],
                        xt_sb[:, k * NT:(k + 1) * NT],
                        1.0,
                        e_sb[:],
                        op0=OP.mult,
                        op1=OP.mult,
                        accum_out=racc_d[:, kd * TILES + t: kd * TILES + t + 1],
                    )
                return (s, t, prod_g)

            def emit_gp_accum(s, t, prod_g):
                racc_a, racc_d = state[s]
                scr_a = scrap.tile([128, NT], bf)
                for k in range(GP_CHUNKS):
                    nc.scalar.activation(
                        scr_a[:],
                        prod_g[:, k * NT:(k + 1) * NT],
                        AF.Copy,
                        accum_out=racc_a[:, k * TILES + t: k * TILES + t + 1],
                    )
                if t == TILES - 1:
                    out_sb = outsbp.tile([128, KCH], f32)
                    nc.vector.reduce_sum(
                        out_sb[:, 0:GP_CHUNKS],
                        racc_a[:].rearrange("p (k t) -> p k t", k=GP_CHUNKS),
                        axis=mybir.AxisListType.X,
                    )
                    nc.vector.reduce_sum(
                        out_sb[:, GP_CHUNKS:KCH],
                        racc_d[:].rearrange("p (k t) -> p k t", k=KCH - GP_CHUNKS),
                        axis=mybir.AxisListType.X,
                    )
                    nc.gpsimd.dma_start(outp[s], out_sb[:])

            def flush_oute(keep):
                while len(pend_oute) > keep:
                    s_, t_, e_ = pend_oute.pop(0)
                    nc.sync.dma_start(oute[s_, t_:t_ + 1, :], e_[0:1, :])

            for g in range(SPC * TILES):
                s, t = divmod(g, TILES)
                if t == 0:
                    # separate accumulators for the ACT and DVE chunk sets —
                    # a shared tile would thread false cross-engine deps
                    racc_a = raccp.tile(
                        [128, GP_CHUNKS * TILES], f32,
                        tag=f"racca{s}", name=f"racca{s}",
                    )
                    racc_d = raccp.tile(
                        [128, (KCH - GP_CHUNKS) * TILES], f32,
                        tag=f"raccd{s}", name=f"raccd{s}",
                    )
                    state[s] = (racc_a, racc_d)
                xt_sb = xtp.tile([128, KCH * NT], bf)
                if g == 0:
                    # quarters: first h-MMs start after 256 KiB, not 1 MiB
                    for q in range(4):
                        nc.sync.dma_start(
                            xt_sb[:, q * 2 * NT:(q + 1) * 2 * NT],
                            xt[s, t, :, q * 2 * NT:(q + 1) * 2 * NT],
                        )
                    # w2rep rides between the early xt pieces (needed ~8us in)
                    nc.sync.dma_start(w2r_sb[:], w2rep[:, :])
                elif g == 1:
                    for hf in range(2):
                        nc.sync.dma_start(
                            xt_sb[:, hf * 4 * NT:(hf + 1) * 4 * NT],
                            xt[s, t, :, hf * 4 * NT:(hf + 1) * 4 * NT],
                        )
                else:
                    nc.sync.dma_start(xt_sb[:], xt[s, t])
                # h-matmuls, k-outer so each arriving xt piece unblocks MMs
                h_ps = hpsp.tile([128, HH * NT], f32, name="h_ps")
                for k in range(KCH):
                    for half in range(HH):
                        nc.tensor.matmul(
                            h_ps[:, half * NT:(half + 1) * NT],
                            w1t_sb[:, k * H + half * 128: k * H + half * 128 + 128],
                            xt_sb[:, k * NT:(k + 1) * NT],
                            start=(k == 0), stop=(k == KCH - 1),
                        )
                # lag-1 pipeline: scores(g-1) MMs ride the PE queue right
                # after h(g); exp(g-1) precedes tanh(g) on ACT so DVE's
                # wsum(g-1) unblocks first; accum copies (g-1) come after
                # tanh(g) so the PE's score path never queues behind them.
                nextE = emit_scores(*prevT) if prevT is not None else None
                tanh_sb = tp.tile([128, HH * NT], bf)
                nc.scalar.activation(tanh_sb[:], h_ps[:], AF.Tanh)
                if prevE is not None:
                    gp = emit_wsum(*prevE)
                    emit_gp_accum(*gp)
                prevE = nextE
                prevT = (s, t, xt_sb, tanh_sb)
                flush_oute(OUTE_LAG)
            # drain: one tile of scores+wsum+accums remains
            prevE2 = emit_scores(*prevT)
            gp = emit_wsum(*prevE)
            emit_gp_accum(*gp)
            gp = emit_wsum(*prevE2)
            emit_gp_accum(*gp)
            flush_oute(0)

    nc.compile()
    return nc


def _get_nc():
    if "nc" not in _NC_CACHE:
        _NC_CACHE["nc"] = _build_nc()
    return _NC_CACHE["nc"]


def _prep_inputs(tiles_embeddings, W1, W2):
    X_bf = tiles_embeddings.astype(BF16)
    # xt[b, t, p, k, j] = X[b, t*NT + j, k*128 + p]
    xt_sw = np.ascontiguousarray(
        X_bf.reshape(B, TILES, NT, KCH, 128).transpose(0, 1, 4, 3, 2)
    ).reshape(B, TILES, 128, KCH * NT)
    # w1t[p, k, h] = W1[h, k*128 + p]
    w1t = np.ascontiguousarray(
        W1.astype(BF16).reshape(H, KCH, 128).transpose(2, 1, 0)
    ).reshape(128, KCH * H)
    # w2rep[p, half*128 + c] = W2[0, half*128 + p]
    w2rep = np.ascontiguousarray(
        np.broadcast_to(
            W2.astype(BF16).reshape(HH, 128).transpose(1, 0)[:, :, None],
            (128, HH, 128),
        )
    ).reshape(128, H)
    return [
        {
            "xt": xt_sw[c * SPC:(c + 1) * SPC],
            "w1t": w1t,
            "w2rep": w2rep,
        }
        for c in range(NCORES)
    ]


def _run(tiles_embeddings, W1, W2, **spmd_kwargs):
    nc = _get_nc()
    in_maps = _prep_inputs(tiles_embeddings, W1, W2)
    res = run_bass_kernel_spmd(nc, in_maps, core_ids=list(range(NCORES)), **spmd_kwargs)
    acc = np.concatenate([r["out"] for r in res.results], axis=0)       # [B, 128, KCH]
    e = np.concatenate([r["oute"] for r in res.results], axis=0)        # [B, TILES, NT]
    l = e.astype(np.float64).sum(axis=(1, 2))                           # [B]
    # out[b, k*128 + p] = acc[b, p, k]
    out = acc.transpose(0, 2, 1).reshape(B, D) / l[:, None]
    return out.astype(np.float32, copy=False), res


def kernel(tiles_embeddings, W1, W2):
    out, _ = _run(
        np.asarray(tiles_embeddings), np.asarray(W1), np.asarray(W2)
    )
    return out


# revision 11
# speedup vs baseline: 1.0480x; 1.0093x over previous
"""AttentionMIL pooling kernel for 8 Trainium2 NeuronCores.

Math (per slide b): h = tanh(X @ W1^T); s = h @ w2; a = softmax(s);
out = a^T @ X, with X [N=8192, D=1024], W1 [H=256, D], w2 [H].

Strategy (v3b — tile-PAIRED elementwise path):
  - Data-parallel over the slide dim: 16 slides / 8 cores = 2 per core.
  - ONE host-swizzled transposed copy of X per core (bf16, [128(d-chunk),
    n-free] tiles) — 32 MiB of HBM traffic per core (DMA floor ~94us).
  - h^T per 512-row n-tile on PE (16 MMs, k-outer), tanh on ACT, scores
    via REPLICATED-w2 stationary (broadcast score row, no max pass since
    |s| <= ||w2||_1), exp on ACT.
  - The weighted sum runs on DVE+ACT at 1x (fused STT / copy-accum), so
    per-op FIXED costs dominate headroom: v3b processes n-tiles in PAIRS
    for the whole scores/wsum path.  One 2 MiB DMA per pair, both tiles'
    score rows land in one PSUM pair tile ([128,2,512] across 2 banks),
    ONE exp over 1024 cols, and every wsum op (DVE STT with accum_out,
    DVE 2x mul, ACT copy-accum) covers a (2,512) free range -> per-pair
    partial sums.  Saves ~0.56us/tile of ACT and ~0.15 of DVE vs v3a,
    putting both under the PE period (~3.92us/tile): the kernel returns
    to PE-bound.
  - GpSimd stays out of the hot loop (its SBUF port is physically shared
    with DVE; tensor ops there poison DVE ~2.5x, measured).
  - l = sum(e): pair score rows DMA'd out (2 KiB, sync queue, emitted two
    pairs late so their waits never block xt dispatches), summed on host;
    out = acc / l on host.
  - Startup: w1t first on the sync HWDGE queue, pair 0 split into 8
    quarter DMAs (first h-MMs ~10us in), 30 warm matmuls bridge the
    preamble so HAM doesn't re-throttle (cold MMs run at half rate).
  - Tail: last pair uses a (5 ACT / 3 DVE)-chunk split (ACT's extra
    chunks accumulate into racc_d's last-pair columns) to even the drain.
"""

import sys

sys.path.insert(0, "/opt/trn_rl_repo")

import numpy as np
import ml_dtypes

import concourse.bacc as bacc
import concourse.tile as tile
from concourse import mybir
from concourse.bass_utils import run_bass_kernel_spmd

BF16 = ml_dtypes.bfloat16
B, N, D, H = 16, 8192, 1024, 256
NCORES = 8
SPC = B // NCORES          # slides per core
NT = 512                   # rows of N per tile
TILES = N // NT
PAIRS = TILES // 2
KCH = D // 128             # d-chunks (contraction chunks of 128)
HH = H // 128              # h halves
GP = 3                     # d-chunks routed DVE(mul) + Scalar(accum-reduce)
GP_LAST = 5                # ...on the final pair of each slide (drain balance)
WARM_MMS = 30
OUTE_LAG = 2

_NC_CACHE = {}


def _build_nc():
    bf = mybir.dt.bfloat16
    f32 = mybir.dt.float32
    AF = mybir.ActivationFunctionType
    OP = mybir.AluOpType

    nc = bacc.Bacc("TRN2", num_devices=NCORES)
    # Host-swizzled transposed layout, pair-major: one fully contiguous
    # 2 MiB region per pair.  xt[s, p, q, (u*KCH + k)*NT + j] =
    #   X[s, (2p+u)*NT + j, k*128 + q]
    xt = nc.declare_dram_parameter(
        "xt", [SPC, PAIRS, 128, 2 * KCH * NT], bf, isOutput=False
    )
    # w1t[q, k*H + h] = W1[h, k*128 + q]
    w1t = nc.declare_dram_parameter("w1t", [128, KCH * H], bf, isOutput=False)
    # w2rep[q, half*128 + c] = W2[0, half*128 + q]  (replicated along c)
    w2rep = nc.declare_dram_parameter("w2rep", [128, H], bf, isOutput=False)
    outp = nc.declare_dram_parameter("out", [SPC, 128, KCH], f32, isOutput=True)
    # e rows per pair, summed on host for the softmax denominator
    oute = nc.declare_dram_parameter("oute", [SPC, PAIRS, 2 * NT], bf, isOutput=True)

    with tile.TileContext(nc) as tc:
        with tc.tile_pool(name="const", bufs=1) as constp, \
             tc.tile_pool(name="xt", bufs=4) as xtp, \
             tc.tile_pool(name="tanh", bufs=3) as tp, \
             tc.tile_pool(name="e128", bufs=4) as ep, \
             tc.tile_pool(name="scr", bufs=2) as scrp, \
             tc.tile_pool(name="scra", bufs=2) as scrap, \
             tc.tile_pool(name="gprod", bufs=3) as gprodp, \
             tc.tile_pool(name="racc", bufs=2) as raccp, \
             tc.tile_pool(name="outsb", bufs=2) as outsbp, \
             tc.tile_pool(name="hps", bufs=2, space="PSUM") as hpsp, \
             tc.tile_pool(name="sps", bufs=1, space="PSUM") as spsp:

            # w1t gates every h-matmul: nothing may precede it in the
            # HWDGE FIFO.
            w1t_sb = constp.tile([128, KCH * H], bf)
            nc.sync.dma_start(w1t_sb[:], w1t[:, :])
            w2r_sb = constp.tile([128, H], bf)

            warm_sb = constp.tile([128, 256], bf)
            nc.gpsimd.memset(warm_sb[:], 0.0)
            warm_ps = hpsp.tile([128, 256], f32, tag="hps")
            # bridge PE activity from the engine preamble (~6.5us) until the
            # first xt quarter lands (~10us): a gap >3.4us re-throttles HAM.
            for _ in range(WARM_MMS):
                nc.tensor.matmul(
                    warm_ps[:, 0:H], warm_sb[:, 0:128], warm_sb[:, 0:H],
                    start=True, stop=True, skip_group_check=True,
                )

            state = {}          # per-slide persistent accumulator tiles
            pend_oute = []      # [(s, p, e_pair)] stores, emitted late

            def h_mms(xt_pair, u, h_ps):
                # k-outer so each arriving xt piece unblocks its 4 MMs
                for k in range(KCH):
                    for half in range(HH):
                        nc.tensor.matmul(
                            h_ps[:, half * NT:(half + 1) * NT],
                            w1t_sb[:, k * H + half * 128: k * H + half * 128 + 128],
                            xt_pair[:, (u * KCH + k) * NT:(u * KCH + k + 1) * NT],
                            start=(k == 0), stop=(k == KCH - 1),
                        )

            def score_mms(s_ps, u, tanh_sb):
                for half in range(HH):
                    nc.tensor.matmul(
                        s_ps[:, u * NT:(u + 1) * NT],
                        w2r_sb[:, half * 128:(half + 1) * 128],
                        tanh_sb[:, half * NT:(half + 1) * NT],
                        start=(half == 0), stop=(half == HH - 1),
                    )

            def emit_exp(s, p, s_ps, xt_pair):
                e_pair = ep.tile([128, 2 * NT], bf)
                nc.scalar.activation(e_pair[:], s_ps[:], AF.Exp)
                pend_oute.append((s, p, e_pair))
                return (s, p, xt_pair, e_pair)

            def emit_wsum(s, p, xt_pair, e_pair):
                # xt_pair free layout: (u, k, j); e_pair: (u, j)
                racc_a, racc_d = state[s]
                gp = GP_LAST if p == PAIRS - 1 else GP
                xt3 = xt_pair[:].rearrange("q (u k j) -> q u k j", u=2, k=KCH)
                e3 = e_pair[:].rearrange("q (u j) -> q u j", u=2)
                # products for the ACT-owned chunks first (2x-rate, one op)
                # so the Scalar engine's accum copies unblock early
                prod = gprodp.tile([128, 2 * GP_LAST * NT], bf)
                prod4 = prod[:].rearrange("q (u k j) -> q u k j", u=2, k=GP_LAST)
                nc.vector.tensor_mul(
                    prod4[:, :, 0:gp, :],
                    xt3[:, :, 0:gp, :],
                    e3[:, :, None, :].broadcast_to([128, 2, gp, NT]),
                )
                scr = scrp.tile([128, 2 * NT], bf)
                for k in range(gp, KCH):
                    nc.vector.scalar_tensor_tensor(
                        scr[:].rearrange("q (u j) -> q u j", u=2),
                        xt3[:, :, k, :],
                        1.0,
                        e3[:],
                        op0=OP.mult,
                        op1=OP.mult,
                        accum_out=racc_d[:, (k - GP) * PAIRS + p:
                                         (k - GP) * PAIRS + p + 1],
                    )
                return (s, p, gp, prod4)

            def emit_accums(s, p, gp, prod4):
                racc_a, racc_d = state[s]
                scr_a = scrap.tile([128, 2 * NT], bf)
                for k in range(gp):
                    # the drain pair's extra ACT chunks land in racc_d's
                    # column for that (chunk, pair) — same spot DVE would use
                    acc = (
                        racc_a[:, k * PAIRS + p: k * PAIRS + p + 1]
                        if k < GP
                        else racc_d[:, (k - GP) * PAIRS + p:
                                    (k - GP) * PAIRS + p + 1]
                    )
                    nc.scalar.activation(
                        scr_a[:].rearrange("q (u j) -> q u j", u=2),
                        prod4[:, :, k, :],
                        AF.Copy,
                        accum_out=acc,
                    )
                if p == PAIRS - 1:
                    out_sb = outsbp.tile([128, KCH], f32)
                    nc.vector.reduce_sum(
                        out_sb[:, 0:GP],
                        racc_a[:].rearrange("q (k p) -> q k p", k=GP),
                        axis=mybir.AxisListType.X,
                    )
                    nc.vector.reduce_sum(
                        out_sb[:, GP:KCH],
                        racc_d[:].rearrange("q (k p) -> q k p", k=KCH - GP),
                        axis=mybir.AxisListType.X,
                    )
                    nc.gpsimd.dma_start(outp[s], out_sb[:])

            def flush_oute(keep):
                while len(pend_oute) > keep:
                    s_, p_, e_ = pend_oute.pop(0)
                    nc.sync.dma_start(oute[s_, p_:p_ + 1, :], e_[0:1, :])

            pend_score = None   # (tanh_sb) of tile 2p+1, scored next iter
            cur_pair = None     # (s, p, s_ps, xt_pair) pair being scored
            prev_wsum = None    # completed pair awaiting wsum+accums

            for g in range(SPC * PAIRS):
                s, p = divmod(g, PAIRS)
                if p == 0:
                    # separate accumulators for the ACT and DVE chunk sets —
                    # a shared tile would thread false cross-engine deps
                    racc_a = raccp.tile(
                        [128, GP * PAIRS], f32, tag=f"racca{s}", name=f"racca{s}",
                    )
                    racc_d = raccp.tile(
                        [128, (KCH - GP) * PAIRS], f32,
                        tag=f"raccd{s}", name=f"raccd{s}",
                    )
                    state[s] = (racc_a, racc_d)
                xt_pair = xtp.tile([128, 2 * KCH * NT], bf)
                src = xt[s, p]
                if g == 0:
                    for q in range(8):
                        nc.sync.dma_start(
                            xt_pair[:, q * 2 * NT:(q + 1) * 2 * NT],
                            src[:, q * 2 * NT:(q + 1) * 2 * NT],
                        )
                    # w2rep rides between the early xt pieces (needed ~9us in)
                    nc.sync.dma_start(w2r_sb[:], w2rep[:, :])
                elif g == 1:
                    for hf in range(2):
                        nc.sync.dma_start(
                            xt_pair[:, hf * KCH * NT:(hf + 1) * KCH * NT],
                            src[:, hf * KCH * NT:(hf + 1) * KCH * NT],
                        )
                else:
                    nc.sync.dma_start(xt_pair[:], src)

                # ---- tile 2p ----
                h_ps = hpsp.tile([128, HH * NT], f32, name="h_ps", tag="hps")
                h_mms(xt_pair, 0, h_ps)
                # score the PREVIOUS pair's right tile, then its exp: exp
                # precedes this pair's tanh in the ACT FIFO so DVE's wsum
                # input is never stuck behind tanh
                ex = None
                if pend_score is not None:
                    ps_, pu_, ptanh = pend_score
                    score_mms(ps_, pu_, ptanh)
                    cs, cp, cps, cxt = cur_pair
                    ex = emit_exp(cs, cp, cps, cxt)
                tanh0 = tp.tile([128, HH * NT], bf)
                nc.scalar.activation(tanh0[:], h_ps[:], AF.Tanh)

                # ---- tile 2p+1 ----
                h_ps1 = hpsp.tile([128, HH * NT], f32, name="h_ps1", tag="hps")
                h_mms(xt_pair, 1, h_ps1)
                s_ps = spsp.tile([128, 2 * NT], f32)
                score_mms(s_ps, 0, tanh0)
                tanh1 = tp.tile([128, HH * NT], bf)
                nc.scalar.activation(tanh1[:], h_ps1[:], AF.Tanh)

                # wsum + accums for the pair scored at the top of this
                # iteration (exp just emitted); accum copies sit after both
                # tanhs in the ACT FIFO so the PE score path never waits
                if prev_wsum is not None:
                    wa = emit_wsum(*prev_wsum)
                    emit_accums(*wa)
                prev_wsum = ex

                pend_score = (s_ps, 1, tanh1)
                cur_pair = (s, p, s_ps, xt_pair)
                flush_oute(OUTE_LAG)

            # drain: score+exp for the final pair, then two pending wsums
            ps_, pu_, ptanh = pend_score
            score_mms(ps_, pu_, ptanh)
            cs, cp, cps, cxt = cur_pair
            ex = emit_exp(cs, cp, cps, cxt)
            wa = emit_wsum(*prev_wsum)
            emit_accums(*wa)
            wa = emit_wsum(*ex)
            emit_accums(*wa)
            flush_oute(0)

    nc.compile()
    return nc


def _get_nc():
    if "nc" not in _NC_CACHE:
        _NC_CACHE["nc"] = _build_nc()
    return _NC_CACHE["nc"]


def _prep_inputs(tiles_embeddings, W1, W2):
    X_bf = tiles_embeddings.astype(BF16)
    # xt[b, p, q, u, k, j] = X[b, (2p+u)*NT + j, k*128 + q]
    xt_sw = np.ascontiguousarray(
        X_bf.reshape(B, PAIRS, 2, NT, KCH, 128).transpose(0, 1, 5, 2, 4, 3)
    ).reshape(B, PAIRS, 128, 2 * KCH * NT)
    # w1t[q, k, h] = W1[h, k*128 + q]
    w1t = np.ascontiguousarray(
        W1.astype(BF16).reshape(H, KCH, 128).transpose(2, 1, 0)
    ).reshape(128, KCH * H)
    # w2rep[q, half*128 + c] = W2[0, half*128 + q]
    w2rep = np.ascontiguousarray(
        np.broadcast_to(
            W2.astype(BF16).reshape(HH, 128).transpose(1, 0)[:, :, None],
            (128, HH, 128),
        )
    ).reshape(128, H)
    return [
        {
            "xt": xt_sw[c * SPC:(c + 1) * SPC],
            "w1t": w1t,
            "w2rep": w2rep,
        }
        for c in range(NCORES)
    ]


def _run(tiles_embeddings, W1, W2, **spmd_kwargs):
    nc = _get_nc()
    in_maps = _prep_inputs(tiles_embeddings, W1, W2)
    res = run_bass_kernel_spmd(nc, in_maps, core_ids=list(range(NCORES)), **spmd_kwargs)
    acc = np.concatenate([r["out"] for r in res.results], axis=0)       # [B, 128, KCH]
    e = np.concatenate([r["oute"] for r in res.results], axis=0)        # [B, PAIRS, 2*NT]
    l = e.astype(np.float64).sum(axis=(1, 2))                           # [B]
    # out[b, k*128 + q] = acc[b, q, k]
    out = acc.transpose(0, 2, 1).reshape(B, D) / l[:, None]
    return out.astype(np.float32, copy=False), res


def kernel(tiles_embeddings, W1, W2):
    out, _ = _run(
        np.asarray(tiles_embeddings), np.asarray(W1), np.asarray(W2)
    )
    return out


# revision 16
# speedup vs baseline: 1.0584x; 1.0100x over previous
"""AttentionMIL pooling kernel for 8 Trainium2 NeuronCores.

Math (per slide b): h = tanh(X @ W1^T); s = h @ w2; a = softmax(s);
out = a^T @ X, with X [N=8192, D=1024], W1 [H=256, D], w2 [H].

Strategy (v3b — tile-PAIRED elementwise path):
  - Data-parallel over the slide dim: 16 slides / 8 cores = 2 per core.
  - ONE host-swizzled transposed copy of X per core (bf16, [128(d-chunk),
    n-free] tiles) — 32 MiB of HBM traffic per core (DMA floor ~94us).
  - h^T per 512-row n-tile on PE (16 MMs, k-outer), tanh on ACT, scores
    via REPLICATED-w2 stationary (broadcast score row, no max pass since
    |s| <= ||w2||_1), exp on ACT.
  - The weighted sum runs on DVE+ACT at 1x (fused STT / copy-accum), so
    per-op FIXED costs dominate headroom: v3b processes n-tiles in PAIRS
    for the whole scores/wsum path.  One 2 MiB DMA per pair, both tiles'
    score rows land in one PSUM pair tile ([128,2,512] across 2 banks),
    ONE exp over 1024 cols, and every wsum op (DVE STT with accum_out,
    DVE 2x mul, ACT copy-accum) covers a (2,512) free range -> per-pair
    partial sums.  Saves ~0.56us/tile of ACT and ~0.15 of DVE vs v3a,
    putting both under the PE period (~3.92us/tile): the kernel returns
    to PE-bound.
  - GpSimd stays out of the hot loop (its SBUF port is physically shared
    with DVE; tensor ops there poison DVE ~2.5x, measured).
  - l = sum(e): pair score rows DMA'd out (2 KiB, sync queue, emitted two
    pairs late so their waits never block xt dispatches), summed on host;
    out = acc / l on host.
  - Startup: w1t first on the sync HWDGE queue, pair 0 split into 8
    quarter DMAs (first h-MMs ~10us in), 30 warm matmuls bridge the
    preamble so HAM doesn't re-throttle (cold MMs run at half rate).
  - Tail: last pair uses a (5 ACT / 3 DVE)-chunk split (ACT's extra
    chunks accumulate into racc_d's last-pair columns) to even the drain.
"""

import sys

sys.path.insert(0, "/opt/trn_rl_repo")

import numpy as np
import ml_dtypes

import concourse.bacc as bacc
import concourse.tile as tile
from concourse import mybir
from concourse.bass_utils import run_bass_kernel_spmd

BF16 = ml_dtypes.bfloat16
B, N, D, H = 16, 8192, 1024, 256
NCORES = 8
SPC = B // NCORES          # slides per core
NT = 512                   # rows of N per tile
TILES = N // NT
PAIRS = TILES // 2
KCH = D // 128             # d-chunks (contraction chunks of 128)
HH = H // 128              # h halves
GP = 3                     # d-chunks routed DVE(mul) + Scalar(accum-reduce)
GP_LAST = 5                # ...on the final pair of each slide (drain balance)
WARM_MMS = 34
OUTE_LAG = 2

_NC_CACHE = {}


def _build_nc():
    bf = mybir.dt.bfloat16
    f32 = mybir.dt.float32
    AF = mybir.ActivationFunctionType
    OP = mybir.AluOpType

    nc = bacc.Bacc("TRN2", num_devices=NCORES)
    # Host-swizzled transposed layout, pair-major: one fully contiguous
    # 2 MiB region per pair.  xt[s, p, q, (u*KCH + k)*NT + j] =
    #   X[s, (2p+u)*NT + j, k*128 + q]
    xt = nc.declare_dram_parameter(
        "xt", [SPC, PAIRS, 128, 2 * KCH * NT], bf, isOutput=False
    )
    # w1t[q, k*H + h] = W1[h, k*128 + q]
    w1t = nc.declare_dram_parameter("w1t", [128, KCH * H], bf, isOutput=False)
    # w2rep[q, half*128 + c] = W2[0, half*128 + q]  (replicated along c)
    w2rep = nc.declare_dram_parameter("w2rep", [128, H], bf, isOutput=False)
    outp = nc.declare_dram_parameter("out", [SPC, 128, KCH], f32, isOutput=True)
    # e rows per pair, summed on host for the softmax denominator
    oute = nc.declare_dram_parameter("oute", [SPC, PAIRS, 2 * NT], bf, isOutput=True)

    with tile.TileContext(nc) as tc:
        with tc.tile_pool(name="const", bufs=1) as constp, \
             tc.tile_pool(name="xt", bufs=4) as xtp, \
             tc.tile_pool(name="tanh", bufs=3) as tp, \
             tc.tile_pool(name="e128", bufs=4) as ep, \
             tc.tile_pool(name="scr", bufs=2) as scrp, \
             tc.tile_pool(name="scra", bufs=2) as scrap, \
             tc.tile_pool(name="gprod", bufs=3) as gprodp, \
             tc.tile_pool(name="racc", bufs=2) as raccp, \
             tc.tile_pool(name="outsb", bufs=2) as outsbp, \
             tc.tile_pool(name="hps", bufs=2, space="PSUM") as hpsp, \
             tc.tile_pool(name="sps", bufs=1, space="PSUM") as spsp:

            # w1t gates every h-matmul: nothing may precede it in the
            # HWDGE FIFO.  Split in two so the first k-chunks' weights land
            # before the full 512 KiB finishes.
            w1t_sb = constp.tile([128, KCH * H], bf)
            nc.sync.dma_start(w1t_sb[:, 0:KCH * H // 2], w1t[:, 0:KCH * H // 2])
            w2r_sb = constp.tile([128, H], bf)

            warm_sb = constp.tile([128, 256], bf)
            # memset on DVE: its preamble ends ~1us before GpSimd's, so the
            # warm matmuls start earlier.
            nc.vector.memset(warm_sb[:], 0.0)
            warm_ps = hpsp.tile([128, 256], f32, tag="hps")
            # bridge PE activity from the engine preamble (~5.5us) until the
            # first xt quarter lands (~9.5us): a gap >3.4us re-throttles HAM
            # and cold MMs run at half rate.  128-col MMs (~107ns cold) give
            # fine-grained coverage.
            for _ in range(WARM_MMS):
                nc.tensor.matmul(
                    warm_ps[:, 0:128], warm_sb[:, 0:128], warm_sb[:, 0:128],
                    start=True, stop=True, skip_group_check=True,
                )

            state = {}          # per-slide persistent accumulator tiles
            pend_oute = []      # [(s, p, e_pair)] stores, emitted late

            def h_mms(xt_pair, u, h_ps, mid=None):
                # k-outer so each arriving xt piece unblocks its 4 MMs; the
                # optional `mid` emits the previous tile's score MMs halfway
                # through, so its exp (and DVE's wsum input) is ready ~2us
                # into the period instead of after the whole h-block.
                for k in range(KCH):
                    if k == KCH // 2 and mid is not None:
                        mid()
                    for half in range(HH):
                        nc.tensor.matmul(
                            h_ps[:, half * NT:(half + 1) * NT],
                            w1t_sb[:, k * H + half * 128: k * H + half * 128 + 128],
                            xt_pair[:, (u * KCH + k) * NT:(u * KCH + k + 1) * NT],
                            start=(k == 0), stop=(k == KCH - 1),
                        )

            def score_mms(s_ps, u, tanh_sb):
                for half in range(HH):
                    nc.tensor.matmul(
                        s_ps[:, u * NT:(u + 1) * NT],
                        w2r_sb[:, half * 128:(half + 1) * 128],
                        tanh_sb[:, half * NT:(half + 1) * NT],
                        start=(half == 0), stop=(half == HH - 1),
                        skip_group_check=True,
                    )

            def emit_exp(s, p, s_ps, xt_pair):
                e_pair = ep.tile([128, 2 * NT], bf)
                nc.scalar.activation(e_pair[:], s_ps[:], AF.Exp)
                pend_oute.append((s, p, e_pair))
                return (s, p, xt_pair, e_pair)

            def emit_wsum(s, p, xt_pair, e_pair):
                # xt_pair free layout: (u, k, j); e_pair: (u, j)
                racc_a, racc_d = state[s]
                gp = GP_LAST if p == PAIRS - 1 else GP
                xt3 = xt_pair[:].rearrange("q (u k j) -> q u k j", u=2, k=KCH)
                e3 = e_pair[:].rearrange("q (u j) -> q u j", u=2)
                # products for the ACT-owned chunks first (2x-rate, one op)
                # so the Scalar engine's accum copies unblock early
                prod = gprodp.tile([128, 2 * GP_LAST * NT], bf)
                prod4 = prod[:].rearrange("q (u k j) -> q u k j", u=2, k=GP_LAST)
                nc.vector.tensor_mul(
                    prod4[:, :, 0:gp, :],
                    xt3[:, :, 0:gp, :],
                    e3[:, :, None, :].broadcast_to([128, 2, gp, NT]),
                )
                scr = scrp.tile([128, 2 * NT], bf)
                for k in range(gp, KCH):
                    nc.vector.scalar_tensor_tensor(
                        scr[:].rearrange("q (u j) -> q u j", u=2),
                        xt3[:, :, k, :],
                        1.0,
                        e3[:],
                        op0=OP.mult,
                        op1=OP.mult,
                        accum_out=racc_d[:, (k - GP) * PAIRS + p:
                                         (k - GP) * PAIRS + p + 1],
                    )
                return (s, p, gp, prod4)

            def emit_accums(s, p, gp, prod4):
                racc_a, racc_d = state[s]
                scr_a = scrap.tile([128, 2 * NT], bf)
                for k in range(gp):
                    # the drain pair's extra ACT chunks land in racc_d's
                    # column for that (chunk, pair) — same spot DVE would use
                    acc = (
                        racc_a[:, k * PAIRS + p: k * PAIRS + p + 1]
                        if k < GP
                        else racc_d[:, (k - GP) * PAIRS + p:
                                    (k - GP) * PAIRS + p + 1]
                    )
                    nc.scalar.activation(
                        scr_a[:].rearrange("q (u j) -> q u j", u=2),
                        prod4[:, :, k, :],
                        AF.Copy,
                        accum_out=acc,
                    )
                if p == PAIRS - 1:
                    out_sb = outsbp.tile([128, KCH], f32)
                    nc.vector.reduce_sum(
                        out_sb[:, 0:GP],
                        racc_a[:].rearrange("q (k p) -> q k p", k=GP),
                        axis=mybir.AxisListType.X,
                    )
                    nc.vector.reduce_sum(
                        out_sb[:, GP:KCH],
                        racc_d[:].rearrange("q (k p) -> q k p", k=KCH - GP),
                        axis=mybir.AxisListType.X,
                    )
                    nc.gpsimd.dma_start(outp[s], out_sb[:])

            def flush_oute(keep):
                while len(pend_oute) > keep:
                    s_, p_, e_ = pend_oute.pop(0)
                    nc.sync.dma_start(oute[s_, p_:p_ + 1, :], e_[0:1, :])

            pend_score = None   # (s_ps, tanh_sb) of tile 2p+1, scored next iter
            cur_pair = None     # (s, p, s_ps, xt_pair) pair being scored

            for g in range(SPC * PAIRS):
                s, p = divmod(g, PAIRS)
                if p == 0:
                    # separate accumulators for the ACT and DVE chunk sets —
                    # a shared tile would thread false cross-engine deps
                    racc_a = raccp.tile(
                        [128, GP * PAIRS], f32, tag=f"racca{s}", name=f"racca{s}",
                    )
                    racc_d = raccp.tile(
                        [128, (KCH - GP) * PAIRS], f32,
                        tag=f"raccd{s}", name=f"raccd{s}",
                    )
                    state[s] = (racc_a, racc_d)
                xt_pair = xtp.tile([128, 2 * KCH * NT], bf)
                src = xt[s, p]
                if g == 0:
                    # quarters interleaved with the w1t second half and
                    # w2rep: first h-MMs start after ~1.3 MiB, not 2.5
                    nc.sync.dma_start(
                        xt_pair[:, 0:2 * NT], src[:, 0:2 * NT],
                    )
                    nc.sync.dma_start(
                        w1t_sb[:, KCH * H // 2:], w1t[:, KCH * H // 2:],
                    )
                    for q in range(1, 8):
                        nc.sync.dma_start(
                            xt_pair[:, q * 2 * NT:(q + 1) * 2 * NT],
                            src[:, q * 2 * NT:(q + 1) * 2 * NT],
                        )
                        if q == 2:
                            nc.sync.dma_start(w2r_sb[:], w2rep[:, :])
                elif g == 1:
                    for hf in range(2):
                        nc.sync.dma_start(
                            xt_pair[:, hf * KCH * NT:(hf + 1) * KCH * NT],
                            src[:, hf * KCH * NT:(hf + 1) * KCH * NT],
                        )
                else:
                    nc.sync.dma_start(xt_pair[:], src)

                # ---- tile 2p ----  (previous pair's last score MMs + exp
                # ride in the middle of this h-block so e(p-1) is ready
                # ~2us into the period)
                exh = [None]

                def mid0():
                    if pend_score is not None:
                        ps_, ptanh = pend_score
                        score_mms(ps_, 1, ptanh)
                        cs, cp, cps, cxt = cur_pair
                        exh[0] = emit_exp(cs, cp, cps, cxt)

                h_ps = hpsp.tile([128, HH * NT], f32, name="h_ps", tag="hps")
                h_mms(xt_pair, 0, h_ps, mid=mid0)
                tanh0 = tp.tile([128, HH * NT], bf)
                nc.scalar.activation(tanh0[:], h_ps[:], AF.Tanh)

                # ---- tile 2p+1 ----
                s_ps = spsp.tile([128, 2 * NT], f32)
                h_ps1 = hpsp.tile([128, HH * NT], f32, name="h_ps1", tag="hps")
                h_mms(xt_pair, 1, h_ps1, mid=lambda: score_mms(s_ps, 0, tanh0))
                tanh1 = tp.tile([128, HH * NT], bf)
                nc.scalar.activation(tanh1[:], h_ps1[:], AF.Tanh)

                # wsum + accums for the previous pair (its exp was emitted in
                # mid0 this iteration); accum copies sit after both tanhs in
                # the ACT FIFO so the PE score path never waits behind them
                if exh[0] is not None:
                    wa = emit_wsum(*exh[0])
                    emit_accums(*wa)

                pend_score = (s_ps, tanh1)
                cur_pair = (s, p, s_ps, xt_pair)
                flush_oute(OUTE_LAG)

            # drain: score+exp+wsum for the final pair only
            ps_, ptanh = pend_score
            score_mms(ps_, 1, ptanh)
            cs, cp, cps, cxt = cur_pair
            ex = emit_exp(cs, cp, cps, cxt)
            wa = emit_wsum(*ex)
            emit_accums(*wa)
            flush_oute(0)

    nc.compile()
    return nc


def _get_nc():
    if "nc" not in _NC_CACHE:
        _NC_CACHE["nc"] = _build_nc()
    return _NC_CACHE["nc"]


def _prep_inputs(tiles_embeddings, W1, W2):
    X_bf = tiles_embeddings.astype(BF16)
    # xt[b, p, q, u, k, j] = X[b, (2p+u)*NT + j, k*128 + q]
    xt_sw = np.ascontiguousarray(
        X_bf.reshape(B, PAIRS, 2, NT, KCH, 128).transpose(0, 1, 5, 2, 4, 3)
    ).reshape(B, PAIRS, 128, 2 * KCH * NT)
    # w1t[q, k, h] = W1[h, k*128 + q]
    w1t = np.ascontiguousarray(
        W1.astype(BF16).reshape(H, KCH, 128).transpose(2, 1, 0)
    ).reshape(128, KCH * H)
    # w2rep[q, half*128 + c] = W2[0, half*128 + q]
    w2rep = np.ascontiguousarray(
        np.broadcast_to(
            W2.astype(BF16).reshape(HH, 128).transpose(1, 0)[:, :, None],
            (128, HH, 128),
        )
    ).reshape(128, H)
    return [
        {
            "xt": xt_sw[c * SPC:(c + 1) * SPC],
            "w1t": w1t,
            "w2rep": w2rep,
        }
        for c in range(NCORES)
    ]


def _run(tiles_embeddings, W1, W2, **spmd_kwargs):
    nc = _get_nc()
    in_maps = _prep_inputs(tiles_embeddings, W1, W2)
    res = run_bass_kernel_spmd(nc, in_maps, core_ids=list(range(NCORES)), **spmd_kwargs)
    acc = np.concatenate([r["out"] for r in res.results], axis=0)       # [B, 128, KCH]
    e = np.concatenate([r["oute"] for r in res.results], axis=0)        # [B, PAIRS, 2*NT]
    l = e.astype(np.float64).sum(axis=(1, 2))                           # [B]
    # out[b, k*128 + q] = acc[b, q, k]
    out = acc.transpose(0, 2, 1).reshape(B, D) / l[:, None]
    return out.astype(np.float32, copy=False), res


def kernel(tiles_embeddings, W1, W2):
    out, _ = _run(
        np.asarray(tiles_embeddings), np.asarray(W1), np.asarray(W2)
    )
    return out
